# revision 17
# baseline (speedup 1.0000x reference)
"""Trainium2 Bass kernel for nn_DividedModel (64 independent MLP towers).

Math (per tower o of O=64):
    h0 = relu(x @ W0[o] + b0[o])         x: [B, 32], W0[o]: [32, 64]
    h1 = relu(h0 @ W1[o] + b1[o])        W1[o]: [64, 64]
    h2 = relu(h1 @ W2[o] + b2[o])        W2[o]: [64, 64]
    out[:, o] = h2 @ W3[o] + b3[o]       W3[o]: [64]

Strategy (v2, PE-array tiling):
  - Data-parallel: batch B=16384 sharded 8 ways (2048 rows/core), params
    replicated; no collectives. Activations kept transposed ([h, batch]).
  - PE tiling (tile_position): L0/L1/L2 run in 64x64 mode - 4 concurrent
    matmuls per 512-cycle slot, one tower per 64x64 tile, ~100% PE
    utilization (the old kernel's block-diagonal pairs got 50%).
  - L3 runs in 128x32 column-tiling mode: 4 concurrent accumulation
    chains, each matmul contributing 2 towers' dot products.
  - All matmul operands fp16 (1 cycle/row; shipped as uint16 through the
    PJRT bridge and bitcast on-chip). PSUM stays fp32 (TRN2 requirement).
  - Evacuation (the real bottleneck, ~1 fp32/lane/cycle from PSUM on each
    of ScalarE/VectorE): one [128, 1024] tensor_scalar(max) op per slot,
    nc.any-routed so the Tile scheduler keeps both engines saturated.
  - Biases: all applied in the evacuation op via per-partition bias
    columns (nonzero-bias build splits each evac into two [128, 512] ops);
    b3 added during the final L3 PSUM copy.

Roofline: evac = 3 layers x 65536 cols / (0.96+1.2 GHz) ~ 95-105 us;
PE ~ 50 us (hidden). Old kernel: 150 us (PE-bound at 50% utilization).
"""

import numpy as np

B, I, O, H = 16384, 32, 64, 64
NCORES = 8
BL = B // NCORES      # 2048 batch rows per core
NB = 512              # batch columns per matmul (one fp32 PSUM bank)
CHUNKS = BL // NB     # 4
NT2 = O // 4          # 16 slots of 4 towers per (chunk, layer)
MM_DT = "fp16"        # matmul operand dtype: fp16 | f32r
SPLIT_PSUM = True     # one PSUM tile per bank so slot matmuls are
                      # independent in the Tile dependency tracker

_CACHE = {}


def _row_of_tower():
    """Map tower id -> partition row of the final L3 PSUM bank.

    L3 matmul j (= 4*p + cq, p in [0,8), cq in [0,4)) reads h2 tile t2=j//2
    bank b=j%2 (towers 4*t2+2b, 4*t2+2b+1) and writes PSUM partitions
    32*cq + 2p (+1)."""
    rows = np.zeros(O, np.int64)
    for j in range(32):
        cq, p = j % 4, j // 4
        t2, b = j // 2, j % 2
        rows[4 * t2 + 2 * b] = 32 * cq + 2 * p
        rows[4 * t2 + 2 * b + 1] = 32 * cq + 2 * p + 1
    return rows


def _build(with_bias: bool = False, reps: int = 1, mm_dt: str = MM_DT,
           profile_mode: str = ""):
    # profile_mode: "" normal | "nomm" evacs only | "noevac" matmuls only
    import concourse.mybir as mybir
    import concourse.tile as tile
    from concourse import bacc

    f32 = mybir.dt.float32
    fp16_via_u16 = mm_dt == "fp16"
    mdt = mybir.dt.uint16 if fp16_via_u16 else mybir.dt.float32r

    def R(ap):
        # fp16 tensors are declared uint16 end-to-end (the axon PJRT
        # bridge rejects F16 transfers); bitcast to f16 at each use.
        return ap.bitcast(mybir.dt.float16) if fp16_via_u16 else ap

    add_op = mybir.AluOpType.add
    max_op = mybir.AluOpType.max

    nc = bacc.Bacc(None, target_bir_lowering=False, debug=False)

    x2_d = nc.dram_tensor("x2", [128, BL], mdt, kind="ExternalInput")
    wl0_d = nc.dram_tensor("wl0", [128, 128 * NT2], mdt, kind="ExternalInput")
    wl1_d = nc.dram_tensor("wl1", [128, 128 * NT2], mdt, kind="ExternalInput")
    wl2_d = nc.dram_tensor("wl2", [128, 128 * NT2], mdt, kind="ExternalInput")
    wl3_d = nc.dram_tensor("wl3", [128, 512], mdt, kind="ExternalInput")
    bb_d = nc.dram_tensor("bb", [128, 97], f32, kind="ExternalInput")
    outT_d = nc.dram_tensor("outT", [128, BL], f32, kind="ExternalOutput")

    hbufs = 48 if fp16_via_u16 else 34

    with tile.TileContext(nc) as tc:
        with (
            tc.tile_pool(name="w", bufs=1) as wpool,
            tc.tile_pool(name="h", bufs=hbufs) as hpool,
            tc.tile_pool(name="ot", bufs=3) as opool,
            tc.tile_pool(name="pp", bufs=(6 if SPLIT_PSUM else 3),
                         space="PSUM") as ppool,
            tc.tile_pool(name="l3", bufs=2, space="PSUM") as l3pool,
        ):
            x2_s = wpool.tile([128, BL], mdt, tag="x2")
            wl0_s = wpool.tile([128, 128 * NT2], mdt, tag="wl0")
            wl1_s = wpool.tile([128, 128 * NT2], mdt, tag="wl1")
            wl2_s = wpool.tile([128, 128 * NT2], mdt, tag="wl2")
            wl3_s = wpool.tile([128, 512], mdt, tag="wl3")
            bb_s = wpool.tile([128, 97], f32, tag="bb")

            # Input loads split across the two DMA-capable queues that do
            # not occupy ScalarE/VectorE, ordered by first use.
            q = 64 * NT2
            nc.sync.dma_start(bb_s[:], bb_d[:])
            nc.sync.dma_start(x2_s[:], x2_d[:])
            nc.gpsimd.dma_start(wl0_s[:, :q], wl0_d[:, :q])
            nc.sync.dma_start(wl0_s[:, q:], wl0_d[:, q:])
            nc.gpsimd.dma_start(wl1_s[:, :q], wl1_d[:, :q])
            nc.sync.dma_start(wl1_s[:, q:], wl1_d[:, q:])
            nc.gpsimd.dma_start(wl2_s[:, :q], wl2_d[:, :q])
            nc.sync.dma_start(wl2_s[:, q:], wl2_d[:, q:])
            nc.gpsimd.dma_start(wl3_s[:], wl3_d[:])

            def alloc_pp(name):
                if SPLIT_PSUM:
                    return [ppool.tile([128, NB], f32, tag="pp", name=name + "a"),
                            ppool.tile([128, NB], f32, tag="pp", name=name + "b")]
                return [ppool.tile([128, 2 * NB], f32, tag="pp", name=name)]

            def evac(dst, pps, layer, t2):
                """dst[SBUF fp16] = relu(pp[PSUM fp32] + bias).

                nc.any: the Tile scheduler routes each op to whichever of
                ScalarE/VectorE is free."""
                if profile_mode == "noevac":
                    # keep the dependency graph valid but make the evac
                    # engines ~free: tiny memsets instead of full relu copies
                    nc.any.memset(R(dst[:]), 0.25)
                    return
                srcs = ([pps[b][:] for b in range(2)] if SPLIT_PSUM
                        else [pps[0][:, NB * b : NB * (b + 1)] for b in range(2)])
                if not with_bias and not SPLIT_PSUM:
                    nc.any.tensor_scalar_max(R(dst[:]), pps[0][:], 0.0)
                elif not with_bias:
                    for b in range(2):
                        nc.any.tensor_scalar_max(
                            R(dst[:, NB * b : NB * (b + 1)]), srcs[b], 0.0)
                else:
                    for b in range(2):
                        col = 1 + 32 * layer + 2 * t2 + b
                        bias_ap = bb_s[:, col : col + 1]
                        nc.any.tensor_scalar(
                            R(dst[:, NB * b : NB * (b + 1)]),
                            srcs[b],
                            bias_ap, 0.0, add_op, max_op,
                        )

            def mm4(pps, wl, rhs_of, t2):
                """One 64x64-tiling slot: 4 concurrent one-tower matmuls.

                Tile (r, c): lhsT = wl[r:r+64, 128*t2 + c ...], rhs from
                partition half r, output -> psum[c:c+64] of bank r//64."""
                # Alternate row groups so each LDWEIGHTS targets PE rows
                # that the previous (still-streaming) matmul is not using,
                # letting the weight load and the streams overlap.
                for r, c in ((0, 0), (64, 0), (0, 64), (64, 64)):
                    if SPLIT_PSUM:
                        pp_ap = pps[r // 64][c : c + 64, 0:NB]
                    else:
                        bk = (r // 64) * NB
                        pp_ap = pps[0][c : c + 64, bk : bk + NB]
                    if profile_mode == "nomm":
                        if r == 0 and c == 0:
                            # minimal writer so the tile has a producer
                            for ppx in pps:
                                nc.tensor.matmul(
                                    ppx[0:64, 0:64], R(wl[0:64, 0:64]),
                                    R(rhs_of(0, 0)[:, 0:64]),
                                    start=True, stop=True,
                                )
                        continue
                    lhsT = wl[r : r + 64, 128 * t2 + c : 128 * t2 + c + 64]
                    nc.tensor.matmul(
                        pp_ap,
                        R(lhsT),
                        R(rhs_of(r, c)),
                        start=True,
                        stop=True,
                        tile_position=(r, c),
                    )

            for _rep in range(reps):
                for ch in range(CHUNKS):
                    cs = NB * ch
                    # ---- L0: h0 = relu(x W0 + b0), tile t2 banks hold
                    # pairs (4t2, 4t2+1), (4t2+2, 4t2+3)
                    h0 = []
                    for t2 in range(NT2):
                        pp = alloc_pp("pp0")
                        mm4(pp, wl0_s, lambda r, c: x2_s[r : r + 64, cs : cs + NB], t2)
                        dst = hpool.tile([128, 2 * NB], mdt, tag="h", name="h0")
                        evac(dst, pp, 0, t2)
                        h0.append(dst)
                    # ---- L1: consumes h0 tile t2 entirely; output banks
                    # hold pairs (4t2, 4t2+2), (4t2+1, 4t2+3)
                    h1 = []
                    for t2 in range(NT2):
                        pp = alloc_pp("pp1")
                        src = h0[t2]

                        def rhs1(r, c, src=src):
                            # partition half r, column bank c//64; which
                            # tower that is is encoded in the wl1 layout
                            b = c // 64
                            return src[r : r + 64, NB * b : NB * (b + 1)]

                        mm4(pp, wl1_s, rhs1, t2)
                        dst = hpool.tile([128, 2 * NB], mdt, tag="h", name="h1")
                        evac(dst, pp, 1, t2)
                        h1.append(dst)
                    # ---- L2: output banks restore pairs (4t2, 4t2+1),
                    # (4t2+2, 4t2+3)
                    h2 = []
                    for t2 in range(NT2):
                        pp = alloc_pp("pp2")
                        src = h1[t2]

                        def rhs2(r, c, src=src):
                            b = c // 64
                            return src[r : r + 64, NB * b : NB * (b + 1)]

                        mm4(pp, wl2_s, rhs2, t2)
                        dst = hpool.tile([128, 2 * NB], mdt, tag="h", name="h2")
                        evac(dst, pp, 2, t2)
                        h2.append(dst)
                    # ---- L3: 4 concurrent column-tiled chains (128x32
                    # mode); chain cq accumulates 8 matmuls, each filling
                    # psum rows 32cq+2p, 32cq+2p+1 of a [16, 512] slice.
                    l3p = l3pool.tile([128, NB], f32, tag="l3", name="l3p")
                    if profile_mode == "nomm":
                        nc.tensor.matmul(
                            l3p[0:16, 0:64], R(wl3_s[:, 0:16]),
                            R(h2[0][:, 0:64]), start=True, stop=True,
                        )
                    for cq in range(4 if profile_mode != "nomm" else 0):
                        for p in range(8):
                            j = 4 * p + cq
                            t2, b = j // 2, j % 2
                            nc.tensor.matmul(
                                l3p[32 * cq : 32 * cq + 16, :],
                                R(wl3_s[:, 16 * j : 16 * (j + 1)]),
                                R(h2[t2][:, NB * b : NB * (b + 1)]),
                                start=(p == 0),
                                stop=(p == 7),
                                tile_position=(0, 32 * cq),
                            )
                    out_sb = opool.tile([128, NB], f32, tag="ot")
                    nc.any.tensor_scalar(
                        out_sb[:], l3p[:], bb_s[:, 0:1], None, add_op
                    )
                    nc.sync.dma_start(outT_d[:, cs : cs + NB], out_sb[:])

    nc.compile()
    return nc


def _prep_weights(W0, b0, W1, b1, W2, b2, W3, b3):
    WL0 = np.zeros((128, 128 * NT2), np.float32)
    WL1 = np.zeros((128, 128 * NT2), np.float32)
    WL2 = np.zeros((128, 128 * NT2), np.float32)
    WL3 = np.zeros((128, 512), np.float32)
    bb = np.zeros((128, 97), np.float32)
    rows = _row_of_tower()
    for o in range(O):
        bb[rows[o], 0] = b3[o]
    for t2 in range(NT2):
        c0 = 128 * t2
        tw = [4 * t2, 4 * t2 + 1, 4 * t2 + 2, 4 * t2 + 3]
        # L0: tile (r, c) -> tower index 2*(r//64) + (c//64)
        WL0[0:32, c0 : c0 + 64] = W0[tw[0]]
        WL0[0:32, c0 + 64 : c0 + 128] = W0[tw[1]]
        WL0[64:96, c0 : c0 + 64] = W0[tw[2]]
        WL0[64:96, c0 + 64 : c0 + 128] = W0[tw[3]]
        # L1: lhsT at parts r holds the tower whose h0 lives at parts r:
        # parts 0-63: towers 4t2 (->c=0), 4t2+2 (->c=64); parts 64-127:
        # towers 4t2+1 (->c=0), 4t2+3 (->c=64)
        WL1[0:64, c0 : c0 + 64] = W1[tw[0]]
        WL1[0:64, c0 + 64 : c0 + 128] = W1[tw[2]]
        WL1[64:128, c0 : c0 + 64] = W1[tw[1]]
        WL1[64:128, c0 + 64 : c0 + 128] = W1[tw[3]]
        # L2: h1 layout: lo half = 4t2 (b0), 4t2+1 (b1); hi = 4t2+2, 4t2+3
        WL2[0:64, c0 : c0 + 64] = W2[tw[0]]
        WL2[0:64, c0 + 64 : c0 + 128] = W2[tw[1]]
        WL2[64:128, c0 : c0 + 64] = W2[tw[2]]
        WL2[64:128, c0 + 64 : c0 + 128] = W2[tw[3]]
        # biases, per (layer, bank) pair columns
        for bk in range(2):
            # h0 banks: (4t2, 4t2+1), (4t2+2, 4t2+3)
            lo, hi = tw[2 * bk], tw[2 * bk + 1]
            bb[0:64, 1 + 2 * t2 + bk] = b0[lo]
            bb[64:128, 1 + 2 * t2 + bk] = b0[hi]
            # h1 banks: (4t2, 4t2+2), (4t2+1, 4t2+3)
            lo, hi = tw[bk], tw[bk + 2]
            bb[0:64, 33 + 2 * t2 + bk] = b1[lo]
            bb[64:128, 33 + 2 * t2 + bk] = b1[hi]
            # h2 banks: (4t2, 4t2+1), (4t2+2, 4t2+3)
            lo, hi = tw[2 * bk], tw[2 * bk + 1]
            bb[0:64, 65 + 2 * t2 + bk] = b2[lo]
            bb[64:128, 65 + 2 * t2 + bk] = b2[hi]
    for j in range(32):
        cq, p = j % 4, j // 4
        t2, b = j // 2, j % 2
        lo, hi = 4 * t2 + 2 * b, 4 * t2 + 2 * b + 1
        WL3[0:64, 16 * j + 2 * p] = W3[lo]
        WL3[64:128, 16 * j + 2 * p + 1] = W3[hi]
    if MM_DT == "fp16":
        cast = lambda a: a.astype(np.float16).view(np.uint16)
    else:
        cast = lambda a: a
    return cast(WL0), cast(WL1), cast(WL2), cast(WL3), bb


def _prep_x(x):
    """Per-core [128, BL] tiles: x^T replicated on all four 32-row groups."""
    xT = np.ascontiguousarray(np.asarray(x, np.float32).T)  # [I, B]
    tiles = []
    for core in range(NCORES):
        sl = xT[:, core * BL : (core + 1) * BL]
        t = np.empty((128, BL), np.float32)
        for r in range(4):
            t[32 * r : 32 * (r + 1)] = sl
        tiles.append(t.astype(np.float16).view(np.uint16)
                     if MM_DT == "fp16" else t)
    return tiles


def kernel(x, W0, b0, W1, b1, W2, b2, W3, b3):
    from concourse.bass_utils import run_bass_kernel_spmd

    x, W0, b0, W1, b1, W2, b2, W3, b3 = (
        np.asarray(a, np.float32) for a in (x, W0, b0, W1, b1, W2, b2, W3, b3)
    )
    with_bias = bool(np.any(b0) or np.any(b1) or np.any(b2))
    key = ("nc", with_bias, MM_DT)
    if key not in _CACHE:
        _CACHE[key] = _build(with_bias, mm_dt=MM_DT)
    nc = _CACHE[key]

    WL0, WL1, WL2, WL3, bb = _prep_weights(W0, b0, W1, b1, W2, b2, W3, b3)
    xts = _prep_x(x)
    in_maps = [
        {"x2": xts[core], "wl0": WL0, "wl1": WL1, "wl2": WL2, "wl3": WL3, "bb": bb}
        for core in range(NCORES)
    ]
    res = run_bass_kernel_spmd(nc, in_maps, core_ids=list(range(NCORES)))
    rows = _row_of_tower()
    out = np.concatenate(
        [r["outT"][rows, :].T for r in res.results], axis=0
    )
    return np.ascontiguousarray(out, np.float32)


if __name__ == "__main__":
    rng = np.random.default_rng(0)
    inputs = {
        "x": rng.standard_normal((B, I), np.float32),
        "W0": rng.standard_normal((O, I, H), np.float32) / np.sqrt(I),
        "b0": np.zeros((O, H), np.float32),
        "W1": rng.standard_normal((O, H, H), np.float32) / np.sqrt(H),
        "b1": np.zeros((O, H), np.float32),
        "W2": rng.standard_normal((O, H, H), np.float32) / np.sqrt(H),
        "b2": np.zeros((O, H), np.float32),
        "W3": rng.standard_normal((O, H), np.float32) / np.sqrt(H),
        "b3": np.zeros((O,), np.float32),
    }
    out = kernel(**inputs)
    print(out.shape, out.dtype, float(np.abs(out).mean()))
